# revision 31
# baseline (speedup 1.0000x reference)
"""Trainium2 Bass kernel for nn_DeformConvNet (deformable conv net).

Sharding: pure data parallelism — batch B=8 across 8 NeuronCores (1 sample
per core); the <1MB parameter set is replicated.

Per-core algorithm (channels on partitions):
  c0:    z = mish(w0.T @ x * s0 + b0)           1x1 conv via fp32r matmul
  9x:    off = conv3x3(z, w_off[i])             9 shifted fp32r matmuls/chunk
         bilinear deform via 3-node hat-mask window (no gathers)
         conv3d tap accumulation into y
  cl:    out = mish(wl.T @ [x; y] * sl + bl)

Layout:
  - "S layout": partition p = (channel n = p%64, image half h = p//64); each
    partition handles 8192 pixels. The torch .view() channel/pixel scramble of
    the offsets becomes a pure stride-2 read after permuting conv output
    channels (even channels -> partitions 0..63, odd -> 64..127).
  - z/samp on a 130x130 zero-padded grid, 67 padded rows per partition
    (h=0: padded rows 0..66 at local r*130; h=1: padded rows 64..129 at local
    (r-64)*130), so both halves share identical access patterns for every
    shifted read and SAME-padding needs no masking.
  - bilinear: cy=clip(gy+off,[0,127]); d=clamp(cy-gy,[-1,1]); row weights
    (Q,1-P-Q,P)=(relu(-d),...,relu(d)); samp = sum_dy M_dy sum_dx N_dx z[.+dy,.+dx].
  - mish(v) = v*t/(t+2), t = e^v*(e^v+2)  (exact algebra; exp on ACT,
    reciprocal_approx_fast on DVE).
"""
import numpy as np

import concourse.bass as bass
import concourse.mybir as mybir
import concourse.tile as tile
from concourse import bacc
from concourse.bass_utils import run_bass_kernel_spmd

F32 = mybir.dt.float32
F32R = mybir.dt.float32r
BF16 = mybir.dt.bfloat16
AF = mybir.ActivationFunctionType
ALU = mybir.AluOpType

B, CH, H, W, CD = 8, 128, 128, 128, 64
HW = H * W            # 16384
HALF = HW // 2        # 8192
GW = 130              # padded grid row width
GROWS = 67            # padded rows stored per partition
GSZ = GROWS * GW      # 8710
FC = 1024             # bilinear chunk (pixels per partition)
NCHUNK = HALF // FC   # 8
EG = 1024             # conv-offset psum group (conv positions) = 2 banks
N_CORES = 8
SAMP_DT = BF16        # samp/conv3d precision


def build_nc():
    nc = bacc.Bacc()

    x_d = nc.dram_tensor("x", [CH, HW], F32, kind="ExternalInput")
    w0_d = nc.dram_tensor("w0d", [CH, 128], F32, kind="ExternalInput")
    s0_d = nc.dram_tensor("s0d", [128, 1], F32, kind="ExternalInput")
    b0_d = nc.dram_tensor("b0d", [128, 1], F32, kind="ExternalInput")
    woff_d = nc.dram_tensor("woff", [9, 128, 9 * 128], F32, kind="ExternalInput")
    zer_d = nc.dram_tensor("zer", [128, GSZ], F32, kind="ExternalInput")
    w3blk_d = nc.dram_tensor("w3blk", [128, 9 * 128], F32, kind="ExternalInput")
    b3_d = nc.dram_tensor("b3d", [128, 1], F32, kind="ExternalInput")
    wlx_d = nc.dram_tensor("wlx", [128, 128], F32, kind="ExternalInput")
    wlyt_d = nc.dram_tensor("wlyt", [CD, 128], F32, kind="ExternalInput")
    wlyb_d = nc.dram_tensor("wlyb", [128, 128], F32, kind="ExternalInput")
    sl_d = nc.dram_tensor("sld", [128, 1], F32, kind="ExternalInput")
    bl_d = nc.dram_tensor("bld", [128, 1], F32, kind="ExternalInput")
    out_d = nc.dram_tensor("out", [CH, HW], F32, kind="ExternalOutput")

    with tile.TileContext(nc) as tc:
        with (
            tc.tile_pool(name="const", bufs=1) as cpool,
            tc.tile_pool(name="big", bufs=1) as bigp,
            tc.tile_pool(name="wt", bufs=2) as wtp,
            tc.tile_pool(name="offp", bufs=3) as offp,
            tc.tile_pool(name="maskp", bufs=2) as mkp,
            tc.tile_pool(name="accp", bufs=2) as acp,
            tc.tile_pool(name="dpool", bufs=2) as dkp,
            tc.tile_pool(name="mishp", bufs=2) as msp,
            tc.tile_pool(name="xin", bufs=2) as xinp,
            tc.tile_pool(name="oup", bufs=2) as oup,
            tc.tile_pool(name="psA", bufs=3, space="PSUM") as psA,
            tc.tile_pool(name="psB", bufs=2, space="PSUM") as psB,
        ):
            # ---- persistent tiles ----
            z_bf = bigp.tile([128, GSZ], BF16, tag="z_bf")
            z_bfo = bigp.tile([128, GSZ], BF16, tag="z_bfo")
            samp_A = bigp.tile([128, GSZ], SAMP_DT, tag="samp_A")
            samp_B = bigp.tile([128, GSZ], SAMP_DT, tag="samp_B")
            y_S = bigp.tile([128, HALF], BF16, tag="y_S")

            w0_t = cpool.tile([CH, 128], F32R)
            s0_t = cpool.tile([128, 1], F32)
            b0_t = cpool.tile([128, 1], F32)
            w3blk_t = cpool.tile([128, 9 * 128], SAMP_DT)
            b3_t = cpool.tile([128, 1], F32)
            wlx_t = cpool.tile([128, 128], F32R)
            wlyt_t = cpool.tile([CD, 128], BF16)
            wlyb_t = cpool.tile([128, 128], BF16)
            sl_t = cpool.tile([128, 1], F32)
            bl_t = cpool.tile([128, 1], F32)

            nc.gpsimd.dma_start(w0_t[:], w0_d[:])
            nc.sync.dma_start(s0_t[:], s0_d[:])
            nc.sync.dma_start(b0_t[:], b0_d[:])
            nc.gpsimd.dma_start(w3blk_t[:], w3blk_d[:])
            nc.sync.dma_start(b3_t[:], b3_d[:])
            nc.gpsimd.dma_start(wlx_t[:], wlx_d[:])
            nc.gpsimd.dma_start(wlyt_t[:], wlyt_d[:])
            nc.gpsimd.dma_start(wlyb_t[:], wlyb_d[:])
            nc.sync.dma_start(sl_t[:], sl_d[:])
            nc.sync.dma_start(bl_t[:], bl_d[:])

            # zero padded grids once (borders stay zero forever)
            nc.gpsimd.memset(z_bf[:], 0.0)
            nc.gpsimd.memset(z_bfo[:], 0.0)
            nc.gpsimd.memset(samp_A[:], 0.0)
            nc.gpsimd.memset(samp_B[:], 0.0)

            def g3(tile_ap, rows, base_row, base_col):
                v = tile_ap.rearrange("p (r c) -> p r c", c=GW)
                return v[:, base_row : base_row + rows, base_col : base_col + 128]

            def mish_from_psum(pst, ncols, scale_ap, bias_ap, writes):
                """mish(scale*psum+bias) -> each (dst_ap, src_slice) in writes."""
                v = msp.tile([128, 512], F32, tag="mv")
                u = msp.tile([128, 512], F32, tag="mu")
                nc.scalar.activation(v[:, :ncols], pst, AF.Identity, bias=bias_ap, scale=scale_ap)
                nc.scalar.activation(u[:, :ncols], pst, AF.Exp, bias=bias_ap, scale=scale_ap)
                t = msp.tile([128, 512], F32, tag="mt")
                nc.vector.scalar_tensor_tensor(t[:, :ncols], u[:, :ncols], 2.0, u[:, :ncols], ALU.add, ALU.mult)
                t2 = msp.tile([128, 512], F32, tag="mt2")
                nc.vector.tensor_scalar(t2[:, :ncols], t[:, :ncols], 2.0, None, ALU.add)
                r = msp.tile([128, 512], F32, tag="mr")
                nc.vector.reciprocal_approx_fast(r[:, :ncols], t2[:, :ncols])
                nc.vector.tensor_tensor(r[:, :ncols], t[:, :ncols], r[:, :ncols], ALU.mult)
                for dst_ap, sl in writes:
                    nc.vector.tensor_tensor(dst_ap, v[sl], r[sl], ALU.mult)

            # ======== c0 ========
            for t in range(32):  # 512-pixel chunks = image rows 4t..4t+3
                    xr = xinp.tile([CH, 512], F32R, tag="xr")
                    nc.gpsimd.dma_start(xr[:], x_d[:, t * 512 : (t + 1) * 512])
                    ps = psB.tile([128, 512], F32, tag="mmps")
                    nc.tensor.matmul(
                        ps[:], w0_t[:], xr[:],
                        start=True, stop=True,
                    )
                    writes = []
                    r0, r1 = 4 * t, 4 * t + 3
                    tr1 = min(r1, 64)
                    if r0 <= tr1:  # top partitions: padded rows 0..66 (image -1..65)
                        nr = tr1 - r0 + 1
                        sl = (slice(0, CD), slice(0, nr * 128))
                        writes.append((g3(z_bf[0:CD], nr, r0 + 1, 1), sl))
                        writes.append((g3(z_bfo[0:CD], nr, r0 + 1, 0), sl))
                    br0 = max(r0, 63)
                    if br0 <= r1:  # bottom: padded rows 64..129 (image 63..128)
                        nr = r1 - br0 + 1
                        sl = (slice(CD, 128), slice((br0 - r0) * 128, (r1 - r0 + 1) * 128))
                        writes.append((g3(z_bf[CD:128], nr, br0 - 63, 1), sl))
                        writes.append((g3(z_bfo[CD:128], nr, br0 - 63, 0), sl))
                    mish_from_psum(ps[:], 512, s0_t[:, 0:1], b0_t[:, 0:1], writes)

            # ======== 9 deformable branches ========
            for i in range(9):
                samp_S = samp_A if i % 2 == 0 else samp_B
                wtop = wtp.tile([CD, 9 * 128], BF16, tag="wtop")
                nc.gpsimd.dma_start(wtop[:], woff_d[i, CD:128, :])
                wbot = wtp.tile([128, 9 * 128], BF16, tag="wbot")
                nc.gpsimd.dma_start(wbot[:], woff_d[i])

                for cc in range(NCHUNK):
                    off_y = offp.tile([128, FC], BF16, tag="offy")
                    off_x = offp.tile([128, FC], BF16, tag="offx")
                    # -- offset conv: 2 psum groups of 8 conv rows --
                    for gg in range(2):
                        g = 2 * cc + gg
                        half_bot = g >= 8
                        pg = psA.tile([128, EG], F32, tag="convps")
                        for tap in range(9):  # tap-outer: adjacent matmuls share a stationary
                            ky, kx = tap // 3, tap % 3
                            for s in range(2):
                                row0 = (8 * g) % 64 + 4 * s
                                if half_bot:
                                    stat = wbot[:, tap * 128 : (tap + 1) * 128]
                                    mov = g3(z_bf[:], 4, row0 + ky, kx)
                                else:
                                    stat = wtop[:, tap * 128 : (tap + 1) * 128]
                                    mov = g3(z_bf[0:CD], 4, row0 + ky, kx)
                                nc.tensor.matmul(
                                    pg[:, s * 512 : (s + 1) * 512], stat, mov,
                                    start=(tap == 0), stop=(tap == 8),
                                )
                        dsty = off_y[:, gg * 512 : (gg + 1) * 512]
                        dstx = off_x[:, gg * 512 : (gg + 1) * 512]
                        nc.scalar.copy(dsty, pg[:, 0::2])
                        nc.scalar.copy(dstx, pg[:, 1::2])

                    # -- bilinear: d = clamp(off, [-1,1]) exactly reproduces
                    # clip(g+off,[0,127])-g except at the literal borders,
                    # which get slice fix-ups below. The whole chunk chain
                    # runs on ONE engine; chunks alternate DVE/POOL so the
                    # two engines pipeline without per-op sem ping-pong. --
                    E = nc.gpsimd if cc % 4 == 3 else nc.vector
                    dyt = dkp.tile([128, FC], BF16, tag="dy")
                    E.tensor_scalar(dyt[:], off_y[:], 1.0, -1.0, ALU.min, ALU.max)
                    if cc == 0:  # image row 0 (top partitions, first 128 cols)
                        E.tensor_scalar(dyt[0:CD, 0:128], off_y[0:CD, 0:128], 0.0, 1.0, ALU.max, ALU.min)
                    if cc == NCHUNK - 1:  # image row 127 (bottom partitions, last 128)
                        E.tensor_scalar(dyt[CD:128, FC - 128 : FC], off_y[CD:128, FC - 128 : FC], 0.0, -1.0, ALU.min, ALU.max)
                    dxt = dkp.tile([128, FC], BF16, tag="dx")
                    E.tensor_scalar(dxt[:], off_x[:], 1.0, -1.0, ALU.min, ALU.max)
                    E.tensor_scalar(dxt[:, 0:FC:128], off_x[:, 0:FC:128], 0.0, 1.0, ALU.max, ALU.min)
                    E.tensor_scalar(dxt[:, 127:FC:128], off_x[:, 127:FC:128], 0.0, -1.0, ALU.min, ALU.max)

                    Py = mkp.tile([128, FC], BF16, tag="Py")
                    Qy = mkp.tile([128, FC], BF16, tag="Qy")
                    E.tensor_scalar(Py[:], dyt[:], 0.0, None, ALU.max)
                    E.tensor_scalar(Qy[:], dyt[:], -1.0, 0.0, ALU.mult, ALU.max)
                    M0y = mkp.tile([128, FC], BF16, tag="M0y")
                    E.tensor_tensor(M0y[:], Py[:], Qy[:], ALU.add)
                    E.tensor_scalar(M0y[:], M0y[:], -1.0, 1.0, ALU.mult, ALU.add)
                    Px = mkp.tile([128, FC], BF16, tag="Px")
                    Qx = mkp.tile([128, FC], BF16, tag="Qx")
                    E.tensor_scalar(Px[:], dxt[:], 0.0, None, ALU.max)
                    E.tensor_scalar(Qx[:], dxt[:], -1.0, 0.0, ALU.mult, ALU.max)
                    M0x = mkp.tile([128, FC], BF16, tag="M0x")
                    E.tensor_tensor(M0x[:], Px[:], Qx[:], ALU.add)
                    E.tensor_scalar(M0x[:], M0x[:], -1.0, 1.0, ALU.mult, ALU.add)

                    NX = {-1: Qx, 0: M0x, 1: Px}
                    MY = {-1: Qy, 0: M0y, 1: Py}
                    row0 = 8 * cc + 1
                    inner = acp.tile([128, FC], BF16, tag="inner")
                    tmp = acp.tile([128, FC], BF16, tag="tmp")
                    acc = acp.tile([128, FC], BF16, tag="acc")
                    for k, ddy in enumerate((-1, 0, 1)):
                        # aligned bf16 reads: x-1 from z_bf@+0, x from z_bfo@+0, x+1 from z_bf@+2
                        zr = lambda ddx: (
                            g3(z_bf[:], 8, row0 + ddy, 0) if ddx == -1
                            else (g3(z_bfo[:], 8, row0 + ddy, 0) if ddx == 0
                                  else g3(z_bf[:], 8, row0 + ddy, 2))
                        )
                        E.tensor_tensor(inner[:], NX[-1][:], zr(-1), ALU.mult)
                        E.tensor_tensor(tmp[:], NX[0][:], zr(0), ALU.mult)
                        E.tensor_tensor(inner[:], inner[:], tmp[:], ALU.add)
                        E.tensor_tensor(tmp[:], NX[1][:], zr(1), ALU.mult)
                        E.tensor_tensor(inner[:], inner[:], tmp[:], ALU.add)
                        if k == 0:
                            E.tensor_tensor(acc[:], MY[ddy][:], inner[:], ALU.mult)
                        elif k == 1:
                            E.tensor_tensor(tmp[:], MY[ddy][:], inner[:], ALU.mult)
                            E.tensor_tensor(acc[:], acc[:], tmp[:], ALU.add)
                        else:
                            E.tensor_tensor(tmp[:], MY[ddy][:], inner[:], ALU.mult)
                            samp_dst = g3(samp_S[:], 8, row0, 1)
                            E.tensor_tensor(samp_dst, acc[:], tmp[:], ALU.add)

                # halo rows between halves (partition shift -> DMA)
                nc.sync.dma_start(
                    samp_S[0:CD, 65 * GW : 66 * GW], samp_S[CD:128, 1 * GW : 2 * GW]
                )
                nc.sync.dma_start(
                    samp_S[CD:128, 0:GW], samp_S[0:CD, 64 * GW : 65 * GW]
                )

                # -- conv3d: block-diagonal stationary computes BOTH image
                # halves per matmul; branch PAIRS accumulate in PSUM (samp_A
                # holds even branch, samp_B odd) before one evacuation  --
                if i % 2 == 1 or i == 8:
                    pair = [(i - 1, samp_A), (i, samp_B)] if i % 2 == 1 else [(i, samp_A)]
                    for q in range(16):  # 512-pixel chunks x both halves
                        pq = psB.tile([128, 512], F32, tag="mmps")
                        for pi, (bi, smp) in enumerate(pair):
                            ky, kx = bi // 3, bi % 3
                            stat = w3blk_t[:, bi * 128 : (bi + 1) * 128]
                            mov = g3(smp[:], 4, 4 * q + ky, kx)
                            nc.tensor.matmul(
                                pq[:, :], stat, mov,
                                start=(pi == 0), stop=(pi == len(pair) - 1),
                            )
                        dst = y_S[:, q * 512 : (q + 1) * 512]
                        if i == 1:
                            nc.scalar.activation(dst, pq[:, :], AF.Identity, bias=b3_t[:, 0:1], scale=1.0)
                        else:
                            nc.vector.tensor_tensor(dst, dst, pq[:, :], ALU.add)

            # ======== cl ========
            for big in range(16):
                for s in range(2):
                    t = big * 2 + s
                    px = t * 512
                    ot = oup.tile([128, 512], F32, tag="ot")
                    xr = xinp.tile([CH, 512], F32R, tag="xr")
                    nc.gpsimd.dma_start(xr[:], x_d[:, px : px + 512])
                    ps = psB.tile([128, 512], F32, tag="mmps")
                    nc.tensor.matmul(
                        ps[:], wlx_t[:], xr[:],
                        start=True, stop=False,
                    )
                    if px < HALF:
                        nc.tensor.matmul(
                            ps[:], wlyt_t[:], y_S[0:CD, px : px + 512],
                            start=False, stop=True,
                        )
                    else:
                        nc.tensor.matmul(
                            ps[:], wlyb_t[:], y_S[:, px - HALF : px - HALF + 512],
                            start=False, stop=True,
                        )
                    mish_from_psum(
                        ps[:], 512, sl_t[:, 0:1], bl_t[:, 0:1],
                        [(ot[:, 0:512], (slice(0, 128), slice(0, 512)))],
                    )
                    nc.sync.dma_start(out_d[:, px : px + 512], ot[:])

    nc.compile()
    return nc


# ---------------- host side ----------------

_NC = None


def _get_nc():
    global _NC
    if _NC is None:
        _NC = build_nc()
    return _NC


def _host_params(w0, s0, b0, w_off, w3d, b3d, wl, sl, bl):
    perm = 2 * (np.arange(128) % 64) + (np.arange(128) // 64)
    w0d = np.ascontiguousarray(w0[:, np.arange(128) % CD]).astype(np.float32)
    s0d = s0[np.arange(128) % CD].reshape(128, 1).astype(np.float32)
    b0d = b0[np.arange(128) % CD].reshape(128, 1).astype(np.float32)

    woff = np.zeros((9, 128, 9 * 128), np.float32)
    for i in range(9):
        for tap in range(9):
            ky, kx = tap // 3, tap % 3
            woff[i, CD:128, tap * 128 : (tap + 1) * 128] = w_off[i, perm, :, ky, kx].T

    w3blk = np.zeros((128, 9 * 128), np.float32)
    for k in range(9):
        w3blk[0:CD, k * 128 : k * 128 + CD] = w3d[:, :, k].T
        w3blk[CD:128, k * 128 + CD : (k + 1) * 128] = w3d[:, :, k].T
    b3dd = b3d[np.arange(128) % CD].reshape(128, 1).astype(np.float32)

    wlx = np.ascontiguousarray(wl[0:128]).astype(np.float32)
    wlyt = np.ascontiguousarray(wl[128:192]).astype(np.float32)
    wlyb = np.zeros((128, 128), np.float32)
    wlyb[CD:128] = wl[128:192]

    return {
        "w0d": w0d, "s0d": s0d, "b0d": b0d, "woff": woff,
        "zer": np.zeros((128, GSZ), np.float32),
        "w3blk": w3blk, "b3d": b3dd,
        "wlx": wlx, "wlyt": wlyt, "wlyb": wlyb,
        "sld": sl.reshape(128, 1).astype(np.float32),
        "bld": bl.reshape(128, 1).astype(np.float32),
    }


def kernel(x, w0, s0, b0, w_off, w3d, b3d, wl, sl, bl, _trace=False):
    x = np.asarray(x, np.float32)
    params = _host_params(
        np.asarray(w0, np.float32), np.asarray(s0, np.float32),
        np.asarray(b0, np.float32), np.asarray(w_off, np.float32),
        np.asarray(w3d, np.float32), np.asarray(b3d, np.float32),
        np.asarray(wl, np.float32), np.asarray(sl, np.float32),
        np.asarray(bl, np.float32),
    )
    in_maps = []
    for b in range(B):
        m = dict(params)
        m["x"] = np.ascontiguousarray(x[b].reshape(CH, HW))
        in_maps.append(m)
    nc = _get_nc()
    res = run_bass_kernel_spmd(nc, in_maps, core_ids=list(range(N_CORES)), trace=_trace)
    out = np.stack([res.results[b]["out"].reshape(CH, H, W) for b in range(B)])
    if _trace:
        return out, res
    return out


# revision 32
# speedup vs baseline: 1.0061x; 1.0061x over previous
"""Trainium2 Bass kernel for nn_DeformConvNet (deformable conv net).

Sharding: pure data parallelism — batch B=8 across 8 NeuronCores (1 sample
per core); the <1MB parameter set is replicated.

Per-core algorithm (channels on partitions):
  c0:    z = mish(w0.T @ x * s0 + b0)           1x1 conv via fp32r matmul
  9x:    off = conv3x3(z, w_off[i])             9 shifted fp32r matmuls/chunk
         bilinear deform via 3-node hat-mask window (no gathers)
         conv3d tap accumulation into y
  cl:    out = mish(wl.T @ [x; y] * sl + bl)

Layout:
  - "S layout": partition p = (channel n = p%64, image half h = p//64); each
    partition handles 8192 pixels. The torch .view() channel/pixel scramble of
    the offsets becomes a pure stride-2 read after permuting conv output
    channels (even channels -> partitions 0..63, odd -> 64..127).
  - z/samp on a 130x130 zero-padded grid, 67 padded rows per partition
    (h=0: padded rows 0..66 at local r*130; h=1: padded rows 64..129 at local
    (r-64)*130), so both halves share identical access patterns for every
    shifted read and SAME-padding needs no masking.
  - bilinear: cy=clip(gy+off,[0,127]); d=clamp(cy-gy,[-1,1]); row weights
    (Q,1-P-Q,P)=(relu(-d),...,relu(d)); samp = sum_dy M_dy sum_dx N_dx z[.+dy,.+dx].
  - mish(v) = v*t/(t+2), t = e^v*(e^v+2)  (exact algebra; exp on ACT,
    reciprocal_approx_fast on DVE).
"""
import numpy as np

import concourse.bass as bass
import concourse.mybir as mybir
import concourse.tile as tile
from concourse import bacc
from concourse.bass_utils import run_bass_kernel_spmd

F32 = mybir.dt.float32
F32R = mybir.dt.float32r
BF16 = mybir.dt.bfloat16
AF = mybir.ActivationFunctionType
ALU = mybir.AluOpType

B, CH, H, W, CD = 8, 128, 128, 128, 64
HW = H * W            # 16384
HALF = HW // 2        # 8192
GW = 130              # padded grid row width
GROWS = 67            # padded rows stored per partition
GSZ = GROWS * GW      # 8710
FC = 1024             # bilinear chunk (pixels per partition)
NCHUNK = HALF // FC   # 8
EG = 1024             # conv-offset psum group (conv positions) = 2 banks
N_CORES = 8
SAMP_DT = BF16        # samp/conv3d precision


def build_nc():
    nc = bacc.Bacc()

    x_d = nc.dram_tensor("x", [CH, HW], F32, kind="ExternalInput")
    w0_d = nc.dram_tensor("w0d", [CH, 128], F32, kind="ExternalInput")
    s0_d = nc.dram_tensor("s0d", [128, 1], F32, kind="ExternalInput")
    b0_d = nc.dram_tensor("b0d", [128, 1], F32, kind="ExternalInput")
    woff_d = nc.dram_tensor("woff", [9, 128, 9 * 128], F32, kind="ExternalInput")
    zer_d = nc.dram_tensor("zer", [128, GSZ], F32, kind="ExternalInput")
    w3blk_d = nc.dram_tensor("w3blk", [128, 9 * 128], F32, kind="ExternalInput")
    b3_d = nc.dram_tensor("b3d", [128, 1], F32, kind="ExternalInput")
    wlx_d = nc.dram_tensor("wlx", [128, 128], F32, kind="ExternalInput")
    wlyt_d = nc.dram_tensor("wlyt", [CD, 128], F32, kind="ExternalInput")
    wlyb_d = nc.dram_tensor("wlyb", [128, 128], F32, kind="ExternalInput")
    sl_d = nc.dram_tensor("sld", [128, 1], F32, kind="ExternalInput")
    bl_d = nc.dram_tensor("bld", [128, 1], F32, kind="ExternalInput")
    out_d = nc.dram_tensor("out", [CH, HW], F32, kind="ExternalOutput")

    with tile.TileContext(nc) as tc:
        with (
            tc.tile_pool(name="const", bufs=1) as cpool,
            tc.tile_pool(name="big", bufs=1) as bigp,
            tc.tile_pool(name="wt", bufs=2) as wtp,
            tc.tile_pool(name="offp", bufs=3) as offp,
            tc.tile_pool(name="maskp", bufs=2) as mkp,
            tc.tile_pool(name="accp", bufs=2) as acp,
            tc.tile_pool(name="dpool", bufs=2) as dkp,
            tc.tile_pool(name="mishp", bufs=2) as msp,
            tc.tile_pool(name="xin", bufs=2) as xinp,
            tc.tile_pool(name="oup", bufs=2) as oup,
            tc.tile_pool(name="psA", bufs=2, space="PSUM") as psA,
            tc.tile_pool(name="psB", bufs=4, space="PSUM") as psB,
        ):
            # ---- persistent tiles ----
            z_bf = bigp.tile([128, GSZ], BF16, tag="z_bf")
            z_bfo = bigp.tile([128, GSZ], BF16, tag="z_bfo")
            samp_A = bigp.tile([128, GSZ], SAMP_DT, tag="samp_A")
            samp_B = bigp.tile([128, GSZ], SAMP_DT, tag="samp_B")
            y_S = bigp.tile([128, HALF], BF16, tag="y_S")

            w0_t = cpool.tile([CH, 128], F32R)
            s0_t = cpool.tile([128, 1], F32)
            b0_t = cpool.tile([128, 1], F32)
            w3blk_t = cpool.tile([128, 9 * 128], SAMP_DT)
            b3_t = cpool.tile([128, 1], F32)
            wlx_t = cpool.tile([128, 128], F32R)
            wlyt_t = cpool.tile([CD, 128], BF16)
            wlyb_t = cpool.tile([128, 128], BF16)
            sl_t = cpool.tile([128, 1], F32)
            bl_t = cpool.tile([128, 1], F32)

            nc.gpsimd.dma_start(w0_t[:], w0_d[:])
            nc.sync.dma_start(s0_t[:], s0_d[:])
            nc.sync.dma_start(b0_t[:], b0_d[:])
            nc.gpsimd.dma_start(w3blk_t[:], w3blk_d[:])
            nc.sync.dma_start(b3_t[:], b3_d[:])
            nc.gpsimd.dma_start(wlx_t[:], wlx_d[:])
            nc.gpsimd.dma_start(wlyt_t[:], wlyt_d[:])
            nc.gpsimd.dma_start(wlyb_t[:], wlyb_d[:])
            nc.sync.dma_start(sl_t[:], sl_d[:])
            nc.sync.dma_start(bl_t[:], bl_d[:])

            # zero padded grids once (borders stay zero forever)
            nc.gpsimd.memset(z_bf[:], 0.0)
            nc.gpsimd.memset(z_bfo[:], 0.0)
            nc.gpsimd.memset(samp_A[:], 0.0)
            nc.gpsimd.memset(samp_B[:], 0.0)

            def g3(tile_ap, rows, base_row, base_col):
                v = tile_ap.rearrange("p (r c) -> p r c", c=GW)
                return v[:, base_row : base_row + rows, base_col : base_col + 128]

            def mish_from_psum(pst, ncols, scale_ap, bias_ap, writes):
                """mish(scale*psum+bias) -> each (dst_ap, src_slice) in writes."""
                v = msp.tile([128, 512], F32, tag="mv")
                u = msp.tile([128, 512], F32, tag="mu")
                nc.scalar.activation(v[:, :ncols], pst, AF.Identity, bias=bias_ap, scale=scale_ap)
                nc.scalar.activation(u[:, :ncols], pst, AF.Exp, bias=bias_ap, scale=scale_ap)
                t = msp.tile([128, 512], F32, tag="mt")
                nc.vector.scalar_tensor_tensor(t[:, :ncols], u[:, :ncols], 2.0, u[:, :ncols], ALU.add, ALU.mult)
                t2 = msp.tile([128, 512], F32, tag="mt2")
                nc.vector.tensor_scalar(t2[:, :ncols], t[:, :ncols], 2.0, None, ALU.add)
                r = msp.tile([128, 512], F32, tag="mr")
                nc.vector.reciprocal_approx_fast(r[:, :ncols], t2[:, :ncols])
                nc.vector.tensor_tensor(r[:, :ncols], t[:, :ncols], r[:, :ncols], ALU.mult)
                for dst_ap, sl in writes:
                    nc.vector.tensor_tensor(dst_ap, v[sl], r[sl], ALU.mult)

            # ======== c0 ========
            for t in range(32):  # 512-pixel chunks = image rows 4t..4t+3
                    xr = xinp.tile([CH, 512], F32R, tag="xr")
                    nc.gpsimd.dma_start(xr[:], x_d[:, t * 512 : (t + 1) * 512])
                    ps = psB.tile([128, 512], F32, tag="mmps")
                    nc.tensor.matmul(
                        ps[:], w0_t[:], xr[:],
                        start=True, stop=True,
                    )
                    writes = []
                    r0, r1 = 4 * t, 4 * t + 3
                    tr1 = min(r1, 64)
                    if r0 <= tr1:  # top partitions: padded rows 0..66 (image -1..65)
                        nr = tr1 - r0 + 1
                        sl = (slice(0, CD), slice(0, nr * 128))
                        writes.append((g3(z_bf[0:CD], nr, r0 + 1, 1), sl))
                        writes.append((g3(z_bfo[0:CD], nr, r0 + 1, 0), sl))
                    br0 = max(r0, 63)
                    if br0 <= r1:  # bottom: padded rows 64..129 (image 63..128)
                        nr = r1 - br0 + 1
                        sl = (slice(CD, 128), slice((br0 - r0) * 128, (r1 - r0 + 1) * 128))
                        writes.append((g3(z_bf[CD:128], nr, br0 - 63, 1), sl))
                        writes.append((g3(z_bfo[CD:128], nr, br0 - 63, 0), sl))
                    mish_from_psum(ps[:], 512, s0_t[:, 0:1], b0_t[:, 0:1], writes)

            # ======== 9 deformable branches ========
            for i in range(9):
                samp_S = samp_A if i % 2 == 0 else samp_B
                wtop = wtp.tile([CD, 9 * 128], BF16, tag="wtop")
                nc.gpsimd.dma_start(wtop[:], woff_d[i, CD:128, :])
                wbot = wtp.tile([128, 9 * 128], BF16, tag="wbot")
                nc.gpsimd.dma_start(wbot[:], woff_d[i])

                for cc in range(NCHUNK):
                    off_y = offp.tile([128, FC], BF16, tag="offy")
                    off_x = offp.tile([128, FC], BF16, tag="offx")
                    # -- offset conv: 2 psum groups of 8 conv rows --
                    for gg in range(2):
                        g = 2 * cc + gg
                        half_bot = g >= 8
                        pg = psA.tile([128, EG], F32, tag="convps")
                        for tap in range(9):  # tap-outer: adjacent matmuls share a stationary
                            ky, kx = tap // 3, tap % 3
                            for s in range(2):
                                row0 = (8 * g) % 64 + 4 * s
                                if half_bot:
                                    stat = wbot[:, tap * 128 : (tap + 1) * 128]
                                    mov = g3(z_bf[:], 4, row0 + ky, kx)
                                else:
                                    stat = wtop[:, tap * 128 : (tap + 1) * 128]
                                    mov = g3(z_bf[0:CD], 4, row0 + ky, kx)
                                nc.tensor.matmul(
                                    pg[:, s * 512 : (s + 1) * 512], stat, mov,
                                    start=(tap == 0), stop=(tap == 8),
                                )
                        dsty = off_y[:, gg * 512 : (gg + 1) * 512]
                        dstx = off_x[:, gg * 512 : (gg + 1) * 512]
                        nc.scalar.copy(dsty, pg[:, 0::2])
                        nc.scalar.copy(dstx, pg[:, 1::2])

                    # -- bilinear: d = clamp(off, [-1,1]) exactly reproduces
                    # clip(g+off,[0,127])-g except at the literal borders,
                    # which get slice fix-ups below. The whole chunk chain
                    # runs on ONE engine; chunks alternate DVE/POOL so the
                    # two engines pipeline without per-op sem ping-pong. --
                    E = nc.gpsimd if cc % 4 == 3 else nc.vector
                    dyt = dkp.tile([128, FC], BF16, tag="dy")
                    E.tensor_scalar(dyt[:], off_y[:], 1.0, -1.0, ALU.min, ALU.max)
                    if cc == 0:  # image row 0 (top partitions, first 128 cols)
                        E.tensor_scalar(dyt[0:CD, 0:128], off_y[0:CD, 0:128], 0.0, 1.0, ALU.max, ALU.min)
                    if cc == NCHUNK - 1:  # image row 127 (bottom partitions, last 128)
                        E.tensor_scalar(dyt[CD:128, FC - 128 : FC], off_y[CD:128, FC - 128 : FC], 0.0, -1.0, ALU.min, ALU.max)
                    dxt = dkp.tile([128, FC], BF16, tag="dx")
                    E.tensor_scalar(dxt[:], off_x[:], 1.0, -1.0, ALU.min, ALU.max)
                    E.tensor_scalar(dxt[:, 0:FC:128], off_x[:, 0:FC:128], 0.0, 1.0, ALU.max, ALU.min)
                    E.tensor_scalar(dxt[:, 127:FC:128], off_x[:, 127:FC:128], 0.0, -1.0, ALU.min, ALU.max)

                    Py = mkp.tile([128, FC], BF16, tag="Py")
                    Qy = mkp.tile([128, FC], BF16, tag="Qy")
                    E.tensor_scalar(Py[:], dyt[:], 0.0, None, ALU.max)
                    E.tensor_scalar(Qy[:], dyt[:], -1.0, 0.0, ALU.mult, ALU.max)
                    M0y = mkp.tile([128, FC], BF16, tag="M0y")
                    E.tensor_tensor(M0y[:], Py[:], Qy[:], ALU.add)
                    E.tensor_scalar(M0y[:], M0y[:], -1.0, 1.0, ALU.mult, ALU.add)
                    Px = mkp.tile([128, FC], BF16, tag="Px")
                    Qx = mkp.tile([128, FC], BF16, tag="Qx")
                    E.tensor_scalar(Px[:], dxt[:], 0.0, None, ALU.max)
                    E.tensor_scalar(Qx[:], dxt[:], -1.0, 0.0, ALU.mult, ALU.max)
                    M0x = mkp.tile([128, FC], BF16, tag="M0x")
                    E.tensor_tensor(M0x[:], Px[:], Qx[:], ALU.add)
                    E.tensor_scalar(M0x[:], M0x[:], -1.0, 1.0, ALU.mult, ALU.add)

                    NX = {-1: Qx, 0: M0x, 1: Px}
                    MY = {-1: Qy, 0: M0y, 1: Py}
                    row0 = 8 * cc + 1
                    inner = acp.tile([128, FC], BF16, tag="inner")
                    tmp = acp.tile([128, FC], BF16, tag="tmp")
                    acc = acp.tile([128, FC], BF16, tag="acc")
                    for k, ddy in enumerate((-1, 0, 1)):
                        # aligned bf16 reads: x-1 from z_bf@+0, x from z_bfo@+0, x+1 from z_bf@+2
                        zr = lambda ddx: (
                            g3(z_bf[:], 8, row0 + ddy, 0) if ddx == -1
                            else (g3(z_bfo[:], 8, row0 + ddy, 0) if ddx == 0
                                  else g3(z_bf[:], 8, row0 + ddy, 2))
                        )
                        E.tensor_tensor(inner[:], NX[-1][:], zr(-1), ALU.mult)
                        E.tensor_tensor(tmp[:], NX[0][:], zr(0), ALU.mult)
                        E.tensor_tensor(inner[:], inner[:], tmp[:], ALU.add)
                        E.tensor_tensor(tmp[:], NX[1][:], zr(1), ALU.mult)
                        E.tensor_tensor(inner[:], inner[:], tmp[:], ALU.add)
                        if k == 0:
                            E.tensor_tensor(acc[:], MY[ddy][:], inner[:], ALU.mult)
                        elif k == 1:
                            E.tensor_tensor(tmp[:], MY[ddy][:], inner[:], ALU.mult)
                            E.tensor_tensor(acc[:], acc[:], tmp[:], ALU.add)
                        else:
                            E.tensor_tensor(tmp[:], MY[ddy][:], inner[:], ALU.mult)
                            samp_dst = g3(samp_S[:], 8, row0, 1)
                            E.tensor_tensor(samp_dst, acc[:], tmp[:], ALU.add)

                # halo rows between halves (partition shift -> DMA)
                nc.sync.dma_start(
                    samp_S[0:CD, 65 * GW : 66 * GW], samp_S[CD:128, 1 * GW : 2 * GW]
                )
                nc.sync.dma_start(
                    samp_S[CD:128, 0:GW], samp_S[0:CD, 64 * GW : 65 * GW]
                )

                # -- conv3d: block-diagonal stationary computes BOTH image
                # halves per matmul; branch PAIRS accumulate in PSUM (samp_A
                # holds even branch, samp_B odd) before one evacuation  --
                if i % 2 == 1 or i == 8:
                    pair = [(i - 1, samp_A), (i, samp_B)] if i % 2 == 1 else [(i, samp_A)]
                    for q in range(16):  # 512-pixel chunks x both halves
                        pq = psB.tile([128, 512], F32, tag="mmps")
                        for pi, (bi, smp) in enumerate(pair):
                            ky, kx = bi // 3, bi % 3
                            stat = w3blk_t[:, bi * 128 : (bi + 1) * 128]
                            mov = g3(smp[:], 4, 4 * q + ky, kx)
                            nc.tensor.matmul(
                                pq[:, :], stat, mov,
                                start=(pi == 0), stop=(pi == len(pair) - 1),
                            )
                        dst = y_S[:, q * 512 : (q + 1) * 512]
                        if i == 1:
                            nc.scalar.activation(dst, pq[:, :], AF.Identity, bias=b3_t[:, 0:1], scale=1.0)
                        else:
                            nc.vector.tensor_tensor(dst, dst, pq[:, :], ALU.add)

            # ======== cl ========
            for big in range(16):
                for s in range(2):
                    t = big * 2 + s
                    px = t * 512
                    ot = oup.tile([128, 512], F32, tag="ot")
                    xr = xinp.tile([CH, 512], F32R, tag="xr")
                    nc.gpsimd.dma_start(xr[:], x_d[:, px : px + 512])
                    ps = psB.tile([128, 512], F32, tag="mmps")
                    nc.tensor.matmul(
                        ps[:], wlx_t[:], xr[:],
                        start=True, stop=False,
                    )
                    if px < HALF:
                        nc.tensor.matmul(
                            ps[:], wlyt_t[:], y_S[0:CD, px : px + 512],
                            start=False, stop=True,
                        )
                    else:
                        nc.tensor.matmul(
                            ps[:], wlyb_t[:], y_S[:, px - HALF : px - HALF + 512],
                            start=False, stop=True,
                        )
                    mish_from_psum(
                        ps[:], 512, sl_t[:, 0:1], bl_t[:, 0:1],
                        [(ot[:, 0:512], (slice(0, 128), slice(0, 512)))],
                    )
                    nc.sync.dma_start(out_d[:, px : px + 512], ot[:])

    nc.compile()
    return nc


# ---------------- host side ----------------

_NC = None


def _get_nc():
    global _NC
    if _NC is None:
        _NC = build_nc()
    return _NC


def _host_params(w0, s0, b0, w_off, w3d, b3d, wl, sl, bl):
    perm = 2 * (np.arange(128) % 64) + (np.arange(128) // 64)
    w0d = np.ascontiguousarray(w0[:, np.arange(128) % CD]).astype(np.float32)
    s0d = s0[np.arange(128) % CD].reshape(128, 1).astype(np.float32)
    b0d = b0[np.arange(128) % CD].reshape(128, 1).astype(np.float32)

    woff = np.zeros((9, 128, 9 * 128), np.float32)
    for i in range(9):
        for tap in range(9):
            ky, kx = tap // 3, tap % 3
            woff[i, CD:128, tap * 128 : (tap + 1) * 128] = w_off[i, perm, :, ky, kx].T

    w3blk = np.zeros((128, 9 * 128), np.float32)
    for k in range(9):
        w3blk[0:CD, k * 128 : k * 128 + CD] = w3d[:, :, k].T
        w3blk[CD:128, k * 128 + CD : (k + 1) * 128] = w3d[:, :, k].T
    b3dd = b3d[np.arange(128) % CD].reshape(128, 1).astype(np.float32)

    wlx = np.ascontiguousarray(wl[0:128]).astype(np.float32)
    wlyt = np.ascontiguousarray(wl[128:192]).astype(np.float32)
    wlyb = np.zeros((128, 128), np.float32)
    wlyb[CD:128] = wl[128:192]

    return {
        "w0d": w0d, "s0d": s0d, "b0d": b0d, "woff": woff,
        "zer": np.zeros((128, GSZ), np.float32),
        "w3blk": w3blk, "b3d": b3dd,
        "wlx": wlx, "wlyt": wlyt, "wlyb": wlyb,
        "sld": sl.reshape(128, 1).astype(np.float32),
        "bld": bl.reshape(128, 1).astype(np.float32),
    }


def kernel(x, w0, s0, b0, w_off, w3d, b3d, wl, sl, bl, _trace=False):
    x = np.asarray(x, np.float32)
    params = _host_params(
        np.asarray(w0, np.float32), np.asarray(s0, np.float32),
        np.asarray(b0, np.float32), np.asarray(w_off, np.float32),
        np.asarray(w3d, np.float32), np.asarray(b3d, np.float32),
        np.asarray(wl, np.float32), np.asarray(sl, np.float32),
        np.asarray(bl, np.float32),
    )
    in_maps = []
    for b in range(B):
        m = dict(params)
        m["x"] = np.ascontiguousarray(x[b].reshape(CH, HW))
        in_maps.append(m)
    nc = _get_nc()
    res = run_bass_kernel_spmd(nc, in_maps, core_ids=list(range(N_CORES)), trace=_trace)
    out = np.stack([res.results[b]["out"].reshape(CH, H, W) for b in range(B)])
    if _trace:
        return out, res
    return out


# revision 33
# speedup vs baseline: 1.1414x; 1.1344x over previous
"""Trainium2 Bass kernel for nn_DeformConvNet (deformable conv net).

Sharding: pure data parallelism — batch B=8 across 8 NeuronCores (1 sample
per core); the <1MB parameter set is replicated.

Per-core algorithm (channels on partitions):
  c0:    z = mish(w0.T @ x * s0 + b0)           1x1 conv via fp32r matmul
  9x:    off = conv3x3(z, w_off[i])             9 shifted fp32r matmuls/chunk
         bilinear deform via 3-node hat-mask window (no gathers)
         conv3d tap accumulation into y
  cl:    out = mish(wl.T @ [x; y] * sl + bl)

Layout:
  - "S layout": partition p = (channel n = p%64, image half h = p//64); each
    partition handles 8192 pixels. The torch .view() channel/pixel scramble of
    the offsets becomes a pure stride-2 read after permuting conv output
    channels (even channels -> partitions 0..63, odd -> 64..127).
  - z/samp on a 130x130 zero-padded grid, 67 padded rows per partition
    (h=0: padded rows 0..66 at local r*130; h=1: padded rows 64..129 at local
    (r-64)*130), so both halves share identical access patterns for every
    shifted read and SAME-padding needs no masking.
  - bilinear: cy=clip(gy+off,[0,127]); d=clamp(cy-gy,[-1,1]); row weights
    (Q,1-P-Q,P)=(relu(-d),...,relu(d)); samp = sum_dy M_dy sum_dx N_dx z[.+dy,.+dx].
  - mish(v) = v*t/(t+2), t = e^v*(e^v+2)  (exact algebra; exp on ACT,
    reciprocal_approx_fast on DVE).
"""
import numpy as np

import concourse.bass as bass
import concourse.mybir as mybir
import concourse.tile as tile
from concourse import bacc
from concourse.bass_utils import run_bass_kernel_spmd

F32 = mybir.dt.float32
F32R = mybir.dt.float32r
BF16 = mybir.dt.bfloat16
AF = mybir.ActivationFunctionType
ALU = mybir.AluOpType

B, CH, H, W, CD = 8, 128, 128, 128, 64
HW = H * W            # 16384
HALF = HW // 2        # 8192
GW = 130              # padded grid row width
GROWS = 67            # padded rows stored per partition
GSZ = GROWS * GW      # 8710
FC = 1024             # bilinear chunk (pixels per partition)
NCHUNK = HALF // FC   # 8
EG = 1024             # conv-offset psum group (conv positions) = 2 banks
N_CORES = 8
SAMP_DT = BF16        # samp/conv3d precision


def build_nc():
    nc = bacc.Bacc()

    x_d = nc.dram_tensor("x", [CH, HW], F32, kind="ExternalInput")
    w0_d = nc.dram_tensor("w0d", [CH, 128], F32, kind="ExternalInput")
    s0_d = nc.dram_tensor("s0d", [128, 1], F32, kind="ExternalInput")
    b0_d = nc.dram_tensor("b0d", [128, 1], F32, kind="ExternalInput")
    woff_d = nc.dram_tensor("woff", [9, 128, 9 * 128], F32, kind="ExternalInput")
    zer_d = nc.dram_tensor("zer", [128, GSZ], F32, kind="ExternalInput")
    w3blk_d = nc.dram_tensor("w3blk", [128, 9 * 128], F32, kind="ExternalInput")
    b3_d = nc.dram_tensor("b3d", [128, 1], F32, kind="ExternalInput")
    wlx_d = nc.dram_tensor("wlx", [128, 128], F32, kind="ExternalInput")
    wlyt_d = nc.dram_tensor("wlyt", [CD, 128], F32, kind="ExternalInput")
    wlyb_d = nc.dram_tensor("wlyb", [128, 128], F32, kind="ExternalInput")
    sl_d = nc.dram_tensor("sld", [128, 1], F32, kind="ExternalInput")
    bl_d = nc.dram_tensor("bld", [128, 1], F32, kind="ExternalInput")
    out_d = nc.dram_tensor("out", [CH, HW], F32, kind="ExternalOutput")

    with tile.TileContext(nc) as tc:
        with (
            tc.tile_pool(name="const", bufs=1) as cpool,
            tc.tile_pool(name="big", bufs=1) as bigp,
            tc.tile_pool(name="wt", bufs=2) as wtp,
            tc.tile_pool(name="offp", bufs=3) as offp,
            tc.tile_pool(name="maskp", bufs=3) as mkp,
            tc.tile_pool(name="accp", bufs=2) as acp,
            tc.tile_pool(name="dpool", bufs=2) as dkp,
            tc.tile_pool(name="mishp", bufs=2) as msp,
            tc.tile_pool(name="xin", bufs=2) as xinp,
            tc.tile_pool(name="oup", bufs=2) as oup,
            tc.tile_pool(name="psA", bufs=2, space="PSUM") as psA,
            tc.tile_pool(name="psB", bufs=4, space="PSUM") as psB,
        ):
            # ---- persistent tiles ----
            z_bf = bigp.tile([128, GSZ], BF16, tag="z_bf")
            z_bfo = bigp.tile([128, GSZ], BF16, tag="z_bfo")
            samp_A = bigp.tile([128, GSZ], SAMP_DT, tag="samp_A")
            samp_B = bigp.tile([128, GSZ], SAMP_DT, tag="samp_B")
            y_S = bigp.tile([128, HALF], BF16, tag="y_S")

            w0_t = cpool.tile([CH, 128], F32R)
            s0_t = cpool.tile([128, 1], F32)
            b0_t = cpool.tile([128, 1], F32)
            w3blk_t = cpool.tile([128, 9 * 128], SAMP_DT)
            b3_t = cpool.tile([128, 1], F32)
            wlx_t = cpool.tile([128, 128], F32R)
            wlyt_t = cpool.tile([CD, 128], BF16)
            wlyb_t = cpool.tile([128, 128], BF16)
            sl_t = cpool.tile([128, 1], F32)
            bl_t = cpool.tile([128, 1], F32)

            nc.gpsimd.dma_start(w0_t[:], w0_d[:])
            nc.sync.dma_start(s0_t[:], s0_d[:])
            nc.sync.dma_start(b0_t[:], b0_d[:])
            nc.gpsimd.dma_start(w3blk_t[:], w3blk_d[:])
            nc.sync.dma_start(b3_t[:], b3_d[:])
            nc.gpsimd.dma_start(wlx_t[:], wlx_d[:])
            nc.gpsimd.dma_start(wlyt_t[:], wlyt_d[:])
            nc.gpsimd.dma_start(wlyb_t[:], wlyb_d[:])
            nc.sync.dma_start(sl_t[:], sl_d[:])
            nc.sync.dma_start(bl_t[:], bl_d[:])

            # zero padded grids once (borders stay zero forever)
            nc.gpsimd.memset(z_bf[:], 0.0)
            nc.gpsimd.memset(z_bfo[:], 0.0)
            nc.gpsimd.memset(samp_A[:], 0.0)
            nc.gpsimd.memset(samp_B[:], 0.0)

            def g3(tile_ap, rows, base_row, base_col):
                v = tile_ap.rearrange("p (r c) -> p r c", c=GW)
                return v[:, base_row : base_row + rows, base_col : base_col + 128]

            def mish_from_psum(pst, ncols, scale_ap, bias_ap, writes):
                """mish(scale*psum+bias) -> each (dst_ap, src_slice) in writes."""
                v = msp.tile([128, 512], F32, tag="mv")
                u = msp.tile([128, 512], F32, tag="mu")
                nc.scalar.activation(v[:, :ncols], pst, AF.Identity, bias=bias_ap, scale=scale_ap)
                nc.scalar.activation(u[:, :ncols], pst, AF.Exp, bias=bias_ap, scale=scale_ap)
                t = msp.tile([128, 512], F32, tag="mt")
                nc.vector.scalar_tensor_tensor(t[:, :ncols], u[:, :ncols], 2.0, u[:, :ncols], ALU.add, ALU.mult)
                t2 = msp.tile([128, 512], F32, tag="mt2")
                nc.vector.tensor_scalar(t2[:, :ncols], t[:, :ncols], 2.0, None, ALU.add)
                r = msp.tile([128, 512], F32, tag="mr")
                nc.vector.reciprocal_approx_fast(r[:, :ncols], t2[:, :ncols])
                nc.vector.tensor_tensor(r[:, :ncols], t[:, :ncols], r[:, :ncols], ALU.mult)
                for dst_ap, sl in writes:
                    nc.vector.tensor_tensor(dst_ap, v[sl], r[sl], ALU.mult)

            # ======== c0 ========
            for t in range(32):  # 512-pixel chunks = image rows 4t..4t+3
                    xr = xinp.tile([CH, 512], F32R, tag="xr")
                    nc.gpsimd.dma_start(xr[:], x_d[:, t * 512 : (t + 1) * 512])
                    ps = psB.tile([128, 512], F32, tag="mmps")
                    nc.tensor.matmul(
                        ps[:], w0_t[:], xr[:],
                        start=True, stop=True,
                    )
                    writes = []
                    r0, r1 = 4 * t, 4 * t + 3
                    tr1 = min(r1, 64)
                    if r0 <= tr1:  # top partitions: padded rows 0..66 (image -1..65)
                        nr = tr1 - r0 + 1
                        sl = (slice(0, CD), slice(0, nr * 128))
                        writes.append((g3(z_bf[0:CD], nr, r0 + 1, 1), sl))
                        writes.append((g3(z_bfo[0:CD], nr, r0 + 1, 0), sl))
                    br0 = max(r0, 63)
                    if br0 <= r1:  # bottom: padded rows 64..129 (image 63..128)
                        nr = r1 - br0 + 1
                        sl = (slice(CD, 128), slice((br0 - r0) * 128, (r1 - r0 + 1) * 128))
                        writes.append((g3(z_bf[CD:128], nr, br0 - 63, 1), sl))
                        writes.append((g3(z_bfo[CD:128], nr, br0 - 63, 0), sl))
                    mish_from_psum(ps[:], 512, s0_t[:, 0:1], b0_t[:, 0:1], writes)

            # ======== 9 deformable branches ========
            for i in range(9):
                samp_S = samp_A if i % 2 == 0 else samp_B
                wtop = wtp.tile([CD, 9 * 128], BF16, tag="wtop")
                nc.gpsimd.dma_start(wtop[:], woff_d[i, CD:128, :])
                wbot = wtp.tile([128, 9 * 128], BF16, tag="wbot")
                nc.gpsimd.dma_start(wbot[:], woff_d[i])

                for cc in range(NCHUNK):
                    off_y = offp.tile([128, FC], BF16, tag="offy")
                    off_x = offp.tile([128, FC], BF16, tag="offx")
                    # -- offset conv: 2 psum groups of 8 conv rows --
                    for gg in range(2):
                        g = 2 * cc + gg
                        half_bot = g >= 8
                        pg = psA.tile([128, EG], F32, tag="convps")
                        for tap in range(9):  # tap-outer: adjacent matmuls share a stationary
                            ky, kx = tap // 3, tap % 3
                            for s in range(2):
                                row0 = (8 * g) % 64 + 4 * s
                                if half_bot:
                                    stat = wbot[:, tap * 128 : (tap + 1) * 128]
                                    mov = g3(z_bf[:], 4, row0 + ky, kx)
                                else:
                                    stat = wtop[:, tap * 128 : (tap + 1) * 128]
                                    mov = g3(z_bf[0:CD], 4, row0 + ky, kx)
                                nc.tensor.matmul(
                                    pg[:, s * 512 : (s + 1) * 512], stat, mov,
                                    start=(tap == 0), stop=(tap == 8),
                                )
                        dsty = off_y[:, gg * 512 : (gg + 1) * 512]
                        dstx = off_x[:, gg * 512 : (gg + 1) * 512]
                        nc.scalar.copy(dsty, pg[:, 0::2])
                        nc.scalar.copy(dstx, pg[:, 1::2])

                    # -- bilinear: d = clamp(off, [-1,1]) exactly reproduces
                    # clip(g+off,[0,127])-g except at the literal borders,
                    # which get slice fix-ups below. The whole chunk chain
                    # runs on ONE engine; chunks alternate DVE/POOL so the
                    # two engines pipeline without per-op sem ping-pong. --
                    E = nc.gpsimd if cc % 4 == 3 else nc.vector
                    dyt = dkp.tile([128, FC], BF16, tag="dy")
                    E.tensor_scalar(dyt[:], off_y[:], 1.0, -1.0, ALU.min, ALU.max)
                    if cc == 0:  # image row 0 (top partitions, first 128 cols)
                        E.tensor_scalar(dyt[0:CD, 0:128], off_y[0:CD, 0:128], 0.0, 1.0, ALU.max, ALU.min)
                    if cc == NCHUNK - 1:  # image row 127 (bottom partitions, last 128)
                        E.tensor_scalar(dyt[CD:128, FC - 128 : FC], off_y[CD:128, FC - 128 : FC], 0.0, -1.0, ALU.min, ALU.max)
                    dxt = dkp.tile([128, FC], BF16, tag="dx")
                    E.tensor_scalar(dxt[:], off_x[:], 1.0, -1.0, ALU.min, ALU.max)
                    E.tensor_scalar(dxt[:, 0:FC:128], off_x[:, 0:FC:128], 0.0, 1.0, ALU.max, ALU.min)
                    E.tensor_scalar(dxt[:, 127:FC:128], off_x[:, 127:FC:128], 0.0, -1.0, ALU.min, ALU.max)

                    Py = mkp.tile([128, FC], BF16, tag="Py")
                    Qy = mkp.tile([128, FC], BF16, tag="Qy")
                    E.tensor_scalar(Py[:], dyt[:], 0.0, None, ALU.max)
                    E.tensor_scalar(Qy[:], dyt[:], -1.0, 0.0, ALU.mult, ALU.max)
                    M0y = mkp.tile([128, FC], BF16, tag="M0y")
                    E.tensor_tensor(M0y[:], Py[:], Qy[:], ALU.add)
                    E.tensor_scalar(M0y[:], M0y[:], -1.0, 1.0, ALU.mult, ALU.add)
                    Px = mkp.tile([128, FC], BF16, tag="Px")
                    Qx = mkp.tile([128, FC], BF16, tag="Qx")
                    E.tensor_scalar(Px[:], dxt[:], 0.0, None, ALU.max)
                    E.tensor_scalar(Qx[:], dxt[:], -1.0, 0.0, ALU.mult, ALU.max)
                    M0x = mkp.tile([128, FC], BF16, tag="M0x")
                    E.tensor_tensor(M0x[:], Px[:], Qx[:], ALU.add)
                    E.tensor_scalar(M0x[:], M0x[:], -1.0, 1.0, ALU.mult, ALU.add)

                    NX = {-1: Qx, 0: M0x, 1: Px}
                    MY = {-1: Qy, 0: M0y, 1: Py}
                    row0 = 8 * cc + 1
                    inner = acp.tile([128, FC], BF16, tag="inner")
                    tmp = acp.tile([128, FC], BF16, tag="tmp")
                    acc = acp.tile([128, FC], BF16, tag="acc")
                    for k, ddy in enumerate((-1, 0, 1)):
                        # aligned bf16 reads: x-1 from z_bf@+0, x from z_bfo@+0, x+1 from z_bf@+2
                        zr = lambda ddx: (
                            g3(z_bf[:], 8, row0 + ddy, 0) if ddx == -1
                            else (g3(z_bfo[:], 8, row0 + ddy, 0) if ddx == 0
                                  else g3(z_bf[:], 8, row0 + ddy, 2))
                        )
                        E.tensor_tensor(inner[:], NX[-1][:], zr(-1), ALU.mult)
                        E.tensor_tensor(tmp[:], NX[0][:], zr(0), ALU.mult)
                        E.tensor_tensor(inner[:], inner[:], tmp[:], ALU.add)
                        E.tensor_tensor(tmp[:], NX[1][:], zr(1), ALU.mult)
                        E.tensor_tensor(inner[:], inner[:], tmp[:], ALU.add)
                        if k == 0:
                            E.tensor_tensor(acc[:], MY[ddy][:], inner[:], ALU.mult)
                        elif k == 1:
                            E.tensor_tensor(tmp[:], MY[ddy][:], inner[:], ALU.mult)
                            E.tensor_tensor(acc[:], acc[:], tmp[:], ALU.add)
                        else:
                            E.tensor_tensor(tmp[:], MY[ddy][:], inner[:], ALU.mult)
                            samp_dst = g3(samp_S[:], 8, row0, 1)
                            E.tensor_tensor(samp_dst, acc[:], tmp[:], ALU.add)

                # halo rows between halves (partition shift -> DMA)
                nc.sync.dma_start(
                    samp_S[0:CD, 65 * GW : 66 * GW], samp_S[CD:128, 1 * GW : 2 * GW]
                )
                nc.sync.dma_start(
                    samp_S[CD:128, 0:GW], samp_S[0:CD, 64 * GW : 65 * GW]
                )

                # -- conv3d: block-diagonal stationary computes BOTH image
                # halves per matmul; branch PAIRS accumulate in PSUM (samp_A
                # holds even branch, samp_B odd) before one evacuation  --
                if i % 2 == 1 or i == 8:
                    pair = [(i - 1, samp_A), (i, samp_B)] if i % 2 == 1 else [(i, samp_A)]
                    for q in range(16):  # 512-pixel chunks x both halves
                        pq = psB.tile([128, 512], F32, tag="mmps")
                        for pi, (bi, smp) in enumerate(pair):
                            ky, kx = bi // 3, bi % 3
                            stat = w3blk_t[:, bi * 128 : (bi + 1) * 128]
                            mov = g3(smp[:], 4, 4 * q + ky, kx)
                            nc.tensor.matmul(
                                pq[:, :], stat, mov,
                                start=(pi == 0), stop=(pi == len(pair) - 1),
                            )
                        dst = y_S[:, q * 512 : (q + 1) * 512]
                        if i == 1:
                            nc.scalar.activation(dst, pq[:, :], AF.Identity, bias=b3_t[:, 0:1], scale=1.0)
                        else:
                            nc.vector.tensor_tensor(dst, dst, pq[:, :], ALU.add)

            # ======== cl ========
            for big in range(16):
                for s in range(2):
                    t = big * 2 + s
                    px = t * 512
                    ot = oup.tile([128, 512], F32, tag="ot")
                    xr = xinp.tile([CH, 512], F32R, tag="xr")
                    nc.gpsimd.dma_start(xr[:], x_d[:, px : px + 512])
                    ps = psB.tile([128, 512], F32, tag="mmps")
                    nc.tensor.matmul(
                        ps[:], wlx_t[:], xr[:],
                        start=True, stop=False,
                    )
                    if px < HALF:
                        nc.tensor.matmul(
                            ps[:], wlyt_t[:], y_S[0:CD, px : px + 512],
                            start=False, stop=True,
                        )
                    else:
                        nc.tensor.matmul(
                            ps[:], wlyb_t[:], y_S[:, px - HALF : px - HALF + 512],
                            start=False, stop=True,
                        )
                    mish_from_psum(
                        ps[:], 512, sl_t[:, 0:1], bl_t[:, 0:1],
                        [(ot[:, 0:512], (slice(0, 128), slice(0, 512)))],
                    )
                    nc.sync.dma_start(out_d[:, px : px + 512], ot[:])

    nc.compile()
    return nc


# ---------------- host side ----------------

_NC = None


def _get_nc():
    global _NC
    if _NC is None:
        _NC = build_nc()
    return _NC


def _host_params(w0, s0, b0, w_off, w3d, b3d, wl, sl, bl):
    perm = 2 * (np.arange(128) % 64) + (np.arange(128) // 64)
    w0d = np.ascontiguousarray(w0[:, np.arange(128) % CD]).astype(np.float32)
    s0d = s0[np.arange(128) % CD].reshape(128, 1).astype(np.float32)
    b0d = b0[np.arange(128) % CD].reshape(128, 1).astype(np.float32)

    woff = np.zeros((9, 128, 9 * 128), np.float32)
    for i in range(9):
        for tap in range(9):
            ky, kx = tap // 3, tap % 3
            woff[i, CD:128, tap * 128 : (tap + 1) * 128] = w_off[i, perm, :, ky, kx].T

    w3blk = np.zeros((128, 9 * 128), np.float32)
    for k in range(9):
        w3blk[0:CD, k * 128 : k * 128 + CD] = w3d[:, :, k].T
        w3blk[CD:128, k * 128 + CD : (k + 1) * 128] = w3d[:, :, k].T
    b3dd = b3d[np.arange(128) % CD].reshape(128, 1).astype(np.float32)

    wlx = np.ascontiguousarray(wl[0:128]).astype(np.float32)
    wlyt = np.ascontiguousarray(wl[128:192]).astype(np.float32)
    wlyb = np.zeros((128, 128), np.float32)
    wlyb[CD:128] = wl[128:192]

    return {
        "w0d": w0d, "s0d": s0d, "b0d": b0d, "woff": woff,
        "zer": np.zeros((128, GSZ), np.float32),
        "w3blk": w3blk, "b3d": b3dd,
        "wlx": wlx, "wlyt": wlyt, "wlyb": wlyb,
        "sld": sl.reshape(128, 1).astype(np.float32),
        "bld": bl.reshape(128, 1).astype(np.float32),
    }


def kernel(x, w0, s0, b0, w_off, w3d, b3d, wl, sl, bl, _trace=False):
    x = np.asarray(x, np.float32)
    params = _host_params(
        np.asarray(w0, np.float32), np.asarray(s0, np.float32),
        np.asarray(b0, np.float32), np.asarray(w_off, np.float32),
        np.asarray(w3d, np.float32), np.asarray(b3d, np.float32),
        np.asarray(wl, np.float32), np.asarray(sl, np.float32),
        np.asarray(bl, np.float32),
    )
    in_maps = []
    for b in range(B):
        m = dict(params)
        m["x"] = np.ascontiguousarray(x[b].reshape(CH, HW))
        in_maps.append(m)
    nc = _get_nc()
    res = run_bass_kernel_spmd(nc, in_maps, core_ids=list(range(N_CORES)), trace=_trace)
    out = np.stack([res.results[b]["out"].reshape(CH, H, W) for b in range(B)])
    if _trace:
        return out, res
    return out
